# revision 43
# baseline (speedup 1.0000x reference)
"""Distributed k-NN retrieval kernel for Trainium2 (8 NeuronCores, SPMD).

Math (per the problem): w_i = 1 / (||q - k_i||^2 + delta) over 1M keys;
top-50 w; out = sum_j values[idx_j] * (w_j / sum_i w_i), shape [1, 64].

Strategy: shard keys row-wise across 8 cores (125000 rows each, padded to
126976 = 2 * 63488). Each core computes all shard NEGATED partial
distances nd = -(||k||^2 - 2 q.k) = -dist + ||q||^2 with bf16 tensor-
engine matmuls accumulating in fp32 PSUM (channels on partitions; top-k
of nd == top-k of w since w = 1/(dist + delta) is strictly decreasing in
dist; the bf16 input rounding perturbs dist by ~0.3 while the rank-50 vs
rank-256 distance margin on randn data is ~6.5, so the candidate
superset is safe, and final weights are recomputed exactly on the host).
Candidates: per 63488-row half, the nd values are spread to [128, 496]
and 3 rounds of DVE max8 / max_index / match_replace extract the top-24
per partition (6144 per core) -- a provable superset of the global
top-50 unless >24 of the top-50 land in one of the 256 (partition, half)
bins. The partial sum of w is computed exactly on-device from the same
spread: dist+delta recovered with one tensor_scalar, then DVE reciprocal
(iterative divide) + reduce. The host gathers candidate indices +
partial sums, recomputes candidate weights exactly in fp32, and does the
final top-50 weighted gather-sum (tiny: O(50k)).

Device-side layout (per core):
  row r in [0, 126976), decomposed r = 63488*b + 3968*s16 + 496*g + f
    b   in {0,1}   : half              (psum slice s = 16*b + s16)
    s16 in [0,16)  : psum-slice within half
    g   in [0,8)   : row group (psum partition)
    f   in [0,496) : psum free column
  channel c = 16*Q + cq (quarter Q in [0,4), cq in [0,16))
  kt[16*g + cq, 15872*Q + 496*s + f] = keys_pad[r, c]   (bf16)
  Each psum slice [8, 496] accumulates 8 matmuls (4 quarters x {-k^2, +2qk}).
  Spread: nd_sp[b][16*g + s16, f] = nd(row) -> candidate (b, p, j, v):
  row = 63488*b + 3968*(p % 16) + 496*(p // 16) + v.
"""

import sys

import numpy as np

for _p in ("/opt/trn_rl_repo", "/opt/pypackages"):
    if _p not in sys.path:
        sys.path.insert(0, _p)

DELTA = 0.001
QUERY_WIDTH = 50
N_TOTAL = 1_000_000
D = 64
NCORES = 8
SHARD = N_TOTAL // NCORES  # 125000
FREE = 496                 # psum free columns per slice
SROWS = 8 * FREE           # 3968 rows per psum slice
HALF = 16 * SROWS          # 63488 rows per half
NBANK = 4                  # candidate/sum banks (8 slices each)
BROWS = 8 * SROWS          # 31744 rows per bank
RPAD = 2 * HALF            # 126976 padded rows per core
W = RPAD * D // 128        # 63488 columns of the transposed layout
QBLK = W // 4              # 15872 cols per channel-quarter block
NTILE = 16                 # DMA tiles, each covers 2 psum slices
NROUND = 3                 # max8 rounds -> top-24 per partition per half
PAD_VAL = 1.0e6


def _build_nc(bias_const: float):
    import concourse.bacc as bacc
    import concourse.mybir as mybir
    import concourse.tile as tile

    nc = bacc.Bacc(None, target_bir_lowering=False)

    kt = nc.dram_tensor("kt", [128, W], mybir.dt.bfloat16, kind="ExternalInput")
    sel8 = nc.dram_tensor("sel8", [128, 8], mybir.dt.bfloat16, kind="ExternalInput")
    # qb[:, Q] = -q (Square bias, quarters 0-1); qb[:, 4+Q] = 2q (STT
    # scalar, quarters 2-3).
    qb = nc.dram_tensor("qb", [128, 8], mybir.dt.float32, kind="ExternalInput")
    out_cand = nc.dram_tensor(
        "out_cand", [128, NBANK * NROUND * 8], mybir.dt.uint32, kind="ExternalOutput"
    )
    out_wacc = nc.dram_tensor(
        "out_wacc", [128, NBANK], mybir.dt.float32, kind="ExternalOutput"
    )

    with tile.TileContext(nc) as tc:
        with (
            tc.tile_pool(name="consts", bufs=1) as consts,
            tc.tile_pool(name="kpool", bufs=3) as kpool,
            tc.tile_pool(name="sqpool", bufs=2) as sqpool,
            tc.tile_pool(name="wpool", bufs=1) as wpool,
            tc.tile_pool(name="psum", bufs=4, space="PSUM") as psum,
        ):
            sel8_sb = consts.tile([128, 8], mybir.dt.bfloat16, tag="sel8")
            qb_sb = consts.tile([128, 8], mybir.dt.float32, tag="qb")
            nc.sync.dma_start(out=sel8_sb[:], in_=sel8[:])
            nc.sync.dma_start(out=qb_sb[:], in_=qb[:])

            BF = 4 * FREE  # 1984 spread columns per bank... (8*496/2)
            SPF = 8 * FREE // 16  # 248 spread cols: [8,3968] -> [128,248]
            # nd_sb[b][g, 496*s8 + f] = -(dist_partial) for row(b, s8, g, f)
            nd_sb = [
                wpool.tile([8, 8 * FREE], mybir.dt.float32, tag=f"nd{b}", name=f"nd{b}")
                for b in range(NBANK)
            ]
            nd_sp = [
                wpool.tile([128, SPF], mybir.dt.float32, tag=f"ndsp{b}", name=f"ndsp{b}")
                for b in range(NBANK)
            ]
            dpd = [
                wpool.tile([128, SPF], mybir.dt.float32, tag=f"dpd{b}", name=f"dpd{b}")
                for b in range(NBANK)
            ]
            wv = [
                wpool.tile([128, SPF], mybir.dt.float32, tag=f"wv{b}", name=f"wv{b}")
                for b in range(NBANK)
            ]
            scr = [
                wpool.tile([128, SPF], mybir.dt.float32, tag=f"scr{i}", name=f"scr{i}")
                for i in range(2)
            ]
            mx = [
                wpool.tile([128, 8], mybir.dt.float32, tag=f"mx{i}", name=f"mx{i}")
                for i in range(NROUND)
            ]
            cand_sb = wpool.tile([128, NBANK * NROUND * 8], mybir.dt.uint32, tag="cand")
            wacc_sb = wpool.tile([128, NBANK], mybir.dt.float32, tag="wacc")
            # PE warmup: ~10us of junk matmuls during the first kt DMA so
            # the HAM clock-gate ramps to full rate before the real stream.
            wrm = wpool.tile([128, FREE], mybir.dt.bfloat16, tag="wrm")
            wps = psum.tile([8, FREE], mybir.dt.float32, tag="wps", name="wps")
            nc.vector.memset(wrm[:], 0.0)
            for _w in range(24):
                nc.tensor.matmul(wps[:], sel8_sb[:], wrm[:], start=True, stop=True)

            for t in range(NTILE):
                # tile t covers psum slices {2t, 2t+1}: per quarter, cols
                # [Q*QBLK + 2*FREE*t, Q*QBLK + 2*FREE*(t+1)).
                ktile = kpool.tile([128, 8 * FREE], mybir.dt.bfloat16, tag="ktile")
                src = kt.rearrange("p (q u) -> p q u", q=4)[
                    :, :, 2 * FREE * t : 2 * FREE * (t + 1)
                ]
                nc.sync.dma_start(
                    out=ktile.rearrange("p (q u) -> p q u", q=4), in_=src
                )

                # Elementwise stage split across ACT and DVE (each alone is
                # 1x-mode-bound at ~70us for the full 8.1M elements):
                #   quarters 0-1 on ACT: (k - q)^2 = Square(k + (-q))
                #   quarters 2-3 on DVE: (k - 2q)*k = k^2 - 2 q.k
                # The missing sum(q^2) over channels 32-63 is folded into
                # the reciprocal bias; top-k ordering is shift-invariant.
                sqk = sqpool.tile([128, 8 * FREE], mybir.dt.bfloat16, tag="sqk")
                for Q in range(2):
                    c0 = 2 * FREE * Q
                    nc.scalar.activation(
                        sqk[:, c0 : c0 + 2 * FREE],
                        ktile[:, c0 : c0 + 2 * FREE],
                        mybir.ActivationFunctionType.Square,
                        bias=qb_sb[:, Q : Q + 1],
                        scale=1.0,
                    )
                for Q in range(2, 4):
                    c0 = 2 * FREE * Q
                    nc.vector.scalar_tensor_tensor(
                        out=sqk[:, c0 : c0 + 2 * FREE],
                        in0=ktile[:, c0 : c0 + 2 * FREE],
                        scalar=qb_sb[:, 4 + Q : 5 + Q],
                        in1=ktile[:, c0 : c0 + 2 * FREE],
                        op0=mybir.AluOpType.subtract,
                        op1=mybir.AluOpType.mult,
                    )

                ps = [
                    psum.tile([8, FREE], mybir.dt.float32, tag="ps", name=f"ps{t}_{_}")
                    for _ in range(2)
                ]
                # nd = -sum((k-q)^2): 4 quarter matmuls per slice, -1 selector.
                for ss in range(2):
                    for Q in range(4):
                        c0 = 2 * FREE * Q + FREE * ss
                        nc.tensor.matmul(
                            ps[ss][:],
                            sel8_sb[:],
                            sqk[:, c0 : c0 + FREE],
                            start=(Q == 0),
                            stop=(Q == 3),
                        )
                # Evacuate -dist_partial from PSUM (fp32).
                for ss in range(2):
                    s = 2 * t + ss
                    b, s8 = divmod(s, 8)
                    nc.scalar.copy(
                        nd_sb[b][:, FREE * s8 : FREE * (s8 + 1)], ps[ss][:]
                    )

                if t % 4 == 3:
                    b = t // 4
                    # Spread [8, 3968] -> [128, 248]:
                    # nd_sp[16g + u, v] = nd_sb[g, 248*u + v]
                    for g in range(8):
                        nc.sync.dma_start(
                            out=nd_sp[b][16 * g : 16 * (g + 1), :],
                            in_=nd_sb[b][g : g + 1, :],
                        )
                    # Candidate path: NROUND rounds of top-8-per-partition.
                    cur = nd_sp[b]
                    for r in range(NROUND):
                        nc.vector.max(mx[r][:], cur[:])
                        nc.vector.max_index(
                            cand_sb[:, 8 * (NROUND * b + r) : 8 * (NROUND * b + r + 1)],
                            mx[r][:],
                            cur[:],
                        )
                        if r < NROUND - 1:
                            nxt = scr[r % 2]
                            nc.vector.match_replace(
                                nxt[:], mx[r][:], cur[:], -1.0e30
                            )
                            cur = nxt
                    # Partial-sum path: w = 1/(bias - nd), exact DVE
                    # reciprocal, per-partition reduce.
                    nc.vector.tensor_scalar(
                        out=dpd[b][:],
                        in0=nd_sp[b][:],
                        scalar1=-1.0,
                        scalar2=bias_const,
                        op0=mybir.AluOpType.mult,
                        op1=mybir.AluOpType.add,
                    )
                    nc.vector.reciprocal(wv[b][:], dpd[b][:])
                    nc.vector.tensor_reduce(
                        out=wacc_sb[:, b : b + 1],
                        in_=wv[b][:],
                        axis=mybir.AxisListType.X,
                        op=mybir.AluOpType.add,
                    )

            nc.sync.dma_start(out=out_cand[:], in_=cand_sb[:])
            nc.sync.dma_start(out=out_wacc[:], in_=wacc_sb[:])

    nc.compile()
    return nc


def _host_inputs(q: np.ndarray, keys: np.ndarray):
    """Build the per-core DRAM input arrays (bf16 keys layout)."""
    import ml_dtypes

    bf16 = ml_dtypes.bfloat16
    # Selector is negated so PSUM accumulates -(||k||^2 - 2 q.k).
    sel8 = np.zeros((128, 8), bf16)
    for j in range(8):
        sel8[16 * j : 16 * (j + 1), j] = bf16(-1.0)
    # qb[p, Q] = -q[16*Q + p%16] (Square bias); qb[p, 4+Q] = +2q[...] (STT).
    qb = np.zeros((128, 8), np.float32)
    for Q in range(4):
        qb[:, Q] = np.tile(-q[16 * Q : 16 * (Q + 1)], 8)
        qb[:, 4 + Q] = np.tile(2.0 * q[16 * Q : 16 * (Q + 1)], 8)

    in_maps = []
    for c in range(NCORES):
        shard = keys[c * SHARD : (c + 1) * SHARD]
        pad = np.full((RPAD, D), PAD_VAL, np.float32)
        pad[:SHARD] = shard
        # [b, s16, g, f, Q, cq] -> [g, cq, Q, b, s16, f] -> [128, W]
        kt = np.ascontiguousarray(
            pad.reshape(2, 16, 8, FREE, 4, 16)
            .transpose(2, 5, 4, 0, 1, 3)
            .reshape(128, W)
            .astype(bf16)
        )
        in_maps.append({"kt": kt, "sel8": sel8, "qb": qb})
    return in_maps


def decode_rows(cand: np.ndarray, b: int) -> np.ndarray:
    """Decode bank b's candidate indices from out_cand [128, NBANK*24] to
    shard rows. Candidate (p, j) with value v in [0, 248):
    spread col c = 248*(p % 16) + v; row = 31744*b + 3968*(c // 496)
    + 496*(p // 16) + (c % 496)."""
    SPF = 8 * FREE // 16  # 248
    v = cand[:, 8 * NROUND * b : 8 * NROUND * (b + 1)].astype(np.int64)  # [128, 24]
    p = np.arange(128)[:, None]
    c = SPF * (p % 16) + v
    rows = BROWS * b + SROWS * (c // FREE) + FREE * (p // 16) + (c % FREE)
    # max_index emits -1 (wrapped to u32 max) for unmatched entries; out-of-
    # range v also lands outside the shard and is filtered by the caller.
    rows[(v < 0) | (v >= SPF)] = RPAD
    return rows.reshape(-1)


def _merge(results, q: np.ndarray, keys: np.ndarray, values: np.ndarray):
    """Host-side gather/unshard: exact top-50 over the candidate superset."""
    S = np.float32(
        sum(np.asarray(r["out_wacc"], np.float64).sum() for r in results)
    )
    g_list = []
    for c, r in enumerate(results):
        cand = np.asarray(r["out_cand"])  # [128, NBANK*24] uint32
        for b in range(NBANK):
            rows = decode_rows(cand, b)
            rows = rows[rows < SHARD]
            g_list.append(c * SHARD + rows)
    g = np.unique(np.concatenate(g_list))
    # exact fp32 recompute of candidate weights
    diff = q[None, :] - keys[g]
    d = (diff * diff).sum(axis=1, dtype=np.float32)
    w = np.float32(1.0) / (d + np.float32(DELTA))
    order = np.lexsort((g, -w))  # descending w, ties by lower global index
    sel = order[:QUERY_WIDTH]
    weights = (w[sel] / S).astype(np.float32)[:, None]
    out = (values[g[sel]] * weights).sum(axis=0, keepdims=True, dtype=np.float32)
    return out.astype(np.float32)


_NC_CACHE: dict = {}


def _get_nc(bias_const: float):
    if bias_const not in _NC_CACHE:
        _NC_CACHE[bias_const] = _build_nc(bias_const)
    return _NC_CACHE[bias_const]


def kernel(key, keys, values):
    from concourse.bass_utils import run_bass_kernel_spmd

    q = np.ascontiguousarray(np.asarray(key, np.float32))
    K = np.ascontiguousarray(np.asarray(keys, np.float32))
    V = np.ascontiguousarray(np.asarray(values, np.float32))
    assert q.shape == (D,) and K.shape == (N_TOTAL, D) and V.shape == (N_TOTAL, D)

    # -nd = dist - sum(q[32:]^2) (quarters 2-3 use the decomposition form),
    # so w = 1/(-nd + DELTA + sum(q[32:]^2)).
    bias_const = float(
        np.float32(DELTA) + (q[32:].astype(np.float32) ** 2).sum(dtype=np.float32)
    )
    nc = _get_nc(bias_const)
    in_maps = _host_inputs(q, K)
    res = run_bass_kernel_spmd(nc, in_maps, list(range(NCORES))).results
    return _merge(res, q, K, V)
